# revision 3
# baseline (speedup 1.0000x reference)
"""Trainium2 Bass kernel for the Potts-discriminator energy model.

Math (reference):
    Xf = X.reshape(B, D)                     # B=64, D=L*N=2688
    j_sum[b]  = sum_ij Xf[b,i] J[i,j] Xf[b,j]
    h_sum[b]  = Xf[b,:] @ H_w + H_b
    energy    = j_sum + h_sum
    out       = sigmoid(energy)
    reg_j     = sum(J**2); reg_h = sum(H_w**2)

Sharding: J is column-sharded across 8 cores (336 cols each).  Core c computes
    G_c = Xf @ J[:, cols_c]                  # [B, 336] via 21 K=128 matmuls
    partial_c[b] = sum_j G_c[b,j] * Xf[b, cols_c][j]
plus sum-of-squares of its J shard.  H_w is appended as a 337th column so
Xf @ H_w falls out of the same matmul.  Host sums the 8 per-sample partials
(the "all-reduce"), adds the bias, and applies the sigmoid on 64 scalars.
"""

import os

import numpy as np

B = 64
L = 128
NS = 21
D = L * NS            # 2688
NCORES = 8
CPC = D // NCORES     # 336 columns of J per core
KT = D // 128         # 21 contraction tiles of 128
NAUG = CPC + 1        # 337: J columns + H_w column
CH = 3                # K-tiles per DMA chunk
NCH = KT // CH        # 7 chunks

_F32 = None           # set lazily (mybir.dt.float32)
_STATE = {}           # holds the compiled Bass module across calls

# Results of the last device run (for test harnesses to inspect profiling).
LAST_RESULTS = None


def _build_module():
    import concourse.bacc as bacc
    import concourse.tile as tile
    from concourse import mybir

    f32 = mybir.dt.float32
    nc = bacc.Bacc("TRN2", target_bir_lowering=False, debug=False,
                   num_devices=NCORES)

    xft_d = nc.dram_tensor("xft", (128, KT, B), f32, kind="ExternalInput").ap()
    jsb_d = nc.dram_tensor("jsb", (128, KT, NAUG), f32, kind="ExternalInput").ap()
    xfc_d = nc.dram_tensor("xfc", (B, CPC), f32, kind="ExternalInput").ap()
    hw2_d = nc.dram_tensor("hw2", (128, KT), f32, kind="ExternalInput").ap()
    out_d = nc.dram_tensor("out", (B, 3), f32, kind="ExternalOutput").ap()

    with tile.TileContext(nc) as tc:
        with (
            tc.tile_pool(name="persist", bufs=1) as persist,
            tc.tile_pool(name="psum", bufs=1, space="PSUM") as psum,
            tc.tile_pool(name="scratch", bufs=2) as scratch,
        ):
            stage = persist.tile([B, 3], f32, tag="stage")
            nc.gpsimd.memset(stage[:], 0.0)
            ones = persist.tile([128, 1], f32, tag="ones")
            nc.gpsimd.memset(ones[:], 1.0)

            xft = persist.tile([128, KT, B], f32, tag="xft")
            nc.sync.dma_start(xft[:], xft_d[:])
            xfc = persist.tile([B, CPC], f32, tag="xfc")
            nc.sync.dma_start(xfc[:], xfc_d[:])
            hw2 = persist.tile([128, KT], f32, tag="hw2")
            nc.sync.dma_start(hw2[:], hw2_d[:])

            chunks = []
            for c in range(NCH):
                jc = persist.tile([128, CH, NAUG], f32, tag=f"jchunk{c}")
                nc.sync.dma_start(jc[:], jsb_d[:, c * CH:(c + 1) * CH, :])
                chunks.append(jc)

            g_ps = psum.tile([B, NAUG], f32, tag="g")
            sq_acc = persist.tile([128, NCH], f32, tag="sq_acc")
            for c in range(NCH):
                for i in range(CH):
                    n = c * CH + i
                    nc.tensor.matmul(
                        g_ps[:],
                        xft[:, n, :],           # lhsT [K=128, M=64]
                        chunks[c][:, i, :],     # rhs  [K=128, N=337]
                        start=(n == 0),
                        stop=(n == KT - 1),
                    )
                sq_out = scratch.tile([128, CH, NAUG], f32, tag="sq_out")
                nc.scalar.activation(
                    sq_out[:], chunks[c][:],
                    mybir.ActivationFunctionType.Square,
                    accum_out=sq_acc[:, c:c + 1],
                )

            # partial_j[b] = sum_j G[b, :336] * Xf_cols[b, :]
            # (tensor_tensor_reduce traps on this runtime — use copy +
            #  mul + reduce on DVE instead)
            g_sb = persist.tile([B, NAUG], f32, tag="g_sb")
            nc.vector.tensor_copy(g_sb[:], g_ps[:])
            dot_out = scratch.tile([B, CPC], f32, tag="dot_out")
            nc.vector.tensor_mul(dot_out[:], g_sb[:, 0:CPC], xfc[:])
            nc.vector.tensor_reduce(
                out=stage[:, 0:1], in_=dot_out[:],
                axis=mybir.AxisListType.X, op=mybir.AluOpType.add,
            )
            # h_pre[b] = (Xf @ H_w)[b]  (column 336 of the augmented matmul)
            nc.vector.tensor_copy(stage[:, 1:2], g_sb[:, CPC:CPC + 1])

            # per-partition sums: [:,0] = sumsq of J shard (incl. H col),
            #                     [:,1] = sumsq of H_w
            regs2 = persist.tile([128, 2], f32, tag="regs2")
            nc.vector.tensor_reduce(
                out=regs2[:, 0:1], in_=sq_acc[:],
                axis=mybir.AxisListType.X, op=mybir.AluOpType.add,
            )
            hsq_out = scratch.tile([128, KT], f32, tag="hsq_out")
            nc.scalar.activation(
                hsq_out[:], hw2[:],
                mybir.ActivationFunctionType.Square,
                accum_out=regs2[:, 1:2],
            )
            # cross-partition reduce: [2,1] = regs2.T @ ones
            reg_ps = psum.tile([2, 1], f32, tag="regps")
            nc.tensor.matmul(reg_ps[:], regs2[:], ones[:], start=True, stop=True)
            nc.vector.tensor_copy(stage[0:2, 2:3], reg_ps[:])

            nc.sync.dma_start(out_d[:], stage[:])

    nc.compile()
    return nc


def _prepare_in_maps(X, J_w, H_w):
    Xf = np.ascontiguousarray(X.reshape(B, D), dtype=np.float32)
    # xft[p, n, m] = Xf[m, n*128 + p]
    xft = np.ascontiguousarray(
        Xf.T.reshape(KT, 128, B).transpose(1, 0, 2))
    hw2 = np.ascontiguousarray(H_w.reshape(KT, 128).T)
    in_maps = []
    for c in range(NCORES):
        cols = slice(c * CPC, (c + 1) * CPC)
        jaug = np.concatenate(
            [J_w[:, cols], H_w[:, None]], axis=1)          # [D, 337]
        jsb = np.ascontiguousarray(
            jaug.reshape(KT, 128, NAUG).transpose(1, 0, 2))  # [128, KT, 337]
        xfc = np.ascontiguousarray(Xf[:, cols])
        in_maps.append({
            "xft": xft, "jsb": jsb.astype(np.float32), "xfc": xfc,
            "hw2": hw2,
        })
    return in_maps


def kernel(X, J_w, H_w, H_b):
    global LAST_RESULTS
    from concourse.bass_utils import run_bass_kernel_spmd

    if "nc" not in _STATE:
        _STATE["nc"] = _build_module()
    nc = _STATE["nc"]

    in_maps = _prepare_in_maps(
        np.asarray(X, dtype=np.float32),
        np.asarray(J_w, dtype=np.float32),
        np.asarray(H_w, dtype=np.float32),
    )
    trace = bool(os.environ.get("KERNEL_TRACE"))
    res = run_bass_kernel_spmd(nc, in_maps, core_ids=list(range(NCORES)),
                               trace=trace)
    LAST_RESULTS = res

    outs = np.stack([r["out"] for r in res.results])       # [8, 64, 3]
    partial_j = outs[:, :, 0].sum(axis=0)                  # [64]
    h_pre = outs[0, :, 1]                                  # [64]
    energy = (partial_j + h_pre + np.float32(np.asarray(H_b).reshape(-1)[0])
              ).astype(np.float32)
    sig = (1.0 / (1.0 + np.exp(-energy.astype(np.float64)))).astype(np.float32)
    reg_h = outs[0, 1, 2]
    reg_j = outs[:, 0, 2].sum() - NCORES * reg_h
    return (sig, energy,
            np.asarray(reg_j, dtype=np.float32),
            np.asarray(reg_h, dtype=np.float32))
